# revision 5
# baseline (speedup 1.0000x reference)
"""Trainium2 Bass kernel for AdaptiveMixtureOfExperts (top-2 SwiGLU MoE).

Strategy (expert-parallel, per the sharding hint):
  - Host computes the tiny router (x @ Wr, top-2, softmax) with jax-on-CPU ops
    that bit-match the reference, then "all-to-all"s tokens by routed expert:
    core e receives exactly the tokens routed to expert e (padded to a common
    capacity C), plus expert e's weights (cast to bf16).
  - Each NeuronCore runs the heavy SwiGLU FFN densely on its gathered tokens:
        hT = W1[e].T @ xgT          (ff on partitions, tokens on free dim)
        uT = (a + b1a) * silu(g + b1g)
        yT = W2[e].T @ uT + b2
    All matmuls keep tokens as the moving/free dimension so no on-chip
    transposes are needed; weights are the stationary operands.
  - Host applies the top-2 combine weights and scatter-adds per-expert outputs
    back into the full [B, S, D] output.

Shapes are hardcoded for the problem instance:
  x:[2,2048,1024] f32, Wr:[1024,8], temp:[1], W1:[8,1024,4096], b1:[8,4096],
  W2:[8,2048,1024], b2:[8,1024].  TOP_K=2, 8 experts on 8 cores.
"""

import os

import numpy as np
import ml_dtypes

D_MODEL = 1024
D_FF = 2048
NUM_EXPERTS = 8
TOP_K = 2
P = 128          # partitions
NT = 512         # token tile (moving free dim per matmul)
N_CORES = 8

_NC_CACHE = {}
LAST_RESULTS = None  # test harness introspection


def _build_nc(C: int, use_silu: bool = True):
    """Build the per-core Bass graph for capacity-C token batches.

    use_silu=False decomposes silu into sigmoid+mul (CoreSim has no Silu).
    """
    import concourse.mybir as mybir
    import concourse.tile as tile
    from concourse import bacc
    from concourse.bass import ts

    f32 = mybir.dt.float32
    bf16 = mybir.dt.bfloat16
    AF = mybir.ActivationFunctionType

    K1 = D_MODEL // P          # 8 k-tiles for matmul1
    K2 = D_FF // P             # 16 k-tiles for matmul2
    NF1 = 2 * D_FF // P        # 32 ff tiles of hT
    NO = D_MODEL // P          # 8 out tiles of yT
    n_tok_tiles = (C + NT - 1) // NT

    nc = bacc.Bacc()
    xgT = nc.declare_dram_parameter("xgT", [D_MODEL, C], bf16, isOutput=False)
    w1 = nc.declare_dram_parameter("w1", [D_MODEL, 2 * D_FF], bf16, isOutput=False)
    w2 = nc.declare_dram_parameter("w2", [D_FF, D_MODEL], bf16, isOutput=False)
    b1t = nc.declare_dram_parameter("b1t", [P, NF1], f32, isOutput=False)
    b2t = nc.declare_dram_parameter("b2t", [P, NO], f32, isOutput=False)
    out = nc.declare_dram_parameter("out", [D_MODEL, C], f32, isOutput=True)

    with tile.TileContext(nc) as tc:
        with (
            tc.tile_pool(name="weights", bufs=1) as wpool,
            tc.tile_pool(name="acts", bufs=2) as upool,
            tc.tile_pool(name="epilogue", bufs=4) as epool,
            tc.tile_pool(name="psA", bufs=2, space="PSUM") as psa_pool,
            tc.tile_pool(name="psG", bufs=2, space="PSUM") as psg_pool,
            tc.tile_pool(name="psY", bufs=2, space="PSUM") as psy_pool,
        ):
            # ---- resident inputs ----
            b1_sb = wpool.tile([P, NF1], f32, name="b1_sb")
            nc.sync.dma_start(out=b1_sb[:], in_=b1t[:])
            b2_sb = wpool.tile([P, NO], f32, name="b2_sb")
            nc.sync.dma_start(out=b2_sb[:], in_=b2t[:])

            xg_sb = []
            w1_sb = []
            for k in range(K1):
                xk = wpool.tile([P, C], bf16, name=f"xg_sb{k}", tag=f"xg{k}")
                nc.sync.dma_start(out=xk[:], in_=xgT[k * P:(k + 1) * P, :])
                xg_sb.append(xk)
                wk = wpool.tile([P, 2 * D_FF], bf16, name=f"w1_sb{k}", tag=f"w1{k}")
                nc.sync.dma_start(out=wk[:], in_=w1[k * P:(k + 1) * P, :])
                w1_sb.append(wk)
            w2_sb = []
            for k in range(K2):
                wk = wpool.tile([P, D_MODEL], bf16, name=f"w2_sb{k}", tag=f"w2{k}")
                nc.sync.dma_start(out=wk[:], in_=w2[k * P:(k + 1) * P, :])
                w2_sb.append(wk)

            # ---- main loop over token tiles ----
            for t in range(n_tok_tiles):
                off = t * NT
                Nt = min(NT, C - off)

                uT = upool.tile([P, K2, NT], bf16, name=f"uT{t}", tag="uT")
                for i in range(K2):
                    ps_a = psa_pool.tile([P, NT], f32, name=f"psa{t}_{i}", tag="psa")
                    for k in range(K1):
                        nc.tensor.matmul(
                            ps_a[:, :Nt],
                            w1_sb[k][:, ts(i, P)],
                            xg_sb[k][:, off:off + Nt],
                            start=(k == 0),
                            stop=(k == K1 - 1),
                        )
                    ps_g = psg_pool.tile([P, NT], f32, name=f"psg{t}_{i}", tag="psg")
                    for k in range(K1):
                        nc.tensor.matmul(
                            ps_g[:, :Nt],
                            w1_sb[k][:, ts(K2 + i, P)],
                            xg_sb[k][:, off:off + Nt],
                            start=(k == 0),
                            stop=(k == K1 - 1),
                        )
                    a_t = epool.tile([P, NT], bf16, name=f"a{t}_{i}", tag="a")
                    nc.scalar.activation(
                        a_t[:, :Nt], ps_a[:, :Nt], AF.Identity,
                        bias=b1_sb[:, i:i + 1],
                    )
                    g_t = epool.tile([P, NT], bf16, name=f"g{t}_{i}", tag="g")
                    if use_silu:
                        nc.scalar.activation(
                            g_t[:, :Nt], ps_g[:, :Nt], AF.Silu,
                            bias=b1_sb[:, K2 + i:K2 + i + 1],
                        )
                    else:
                        s_t = epool.tile([P, NT], bf16, name=f"s{t}_{i}", tag="s")
                        nc.scalar.activation(
                            s_t[:, :Nt], ps_g[:, :Nt], AF.Sigmoid,
                            bias=b1_sb[:, K2 + i:K2 + i + 1],
                        )
                        gb_t = epool.tile([P, NT], bf16, name=f"gb{t}_{i}", tag="gb")
                        nc.scalar.activation(
                            gb_t[:, :Nt], ps_g[:, :Nt], AF.Identity,
                            bias=b1_sb[:, K2 + i:K2 + i + 1],
                        )
                        nc.vector.tensor_mul(g_t[:, :Nt], gb_t[:, :Nt], s_t[:, :Nt])
                    nc.vector.tensor_mul(uT[:, i, :Nt], a_t[:, :Nt], g_t[:, :Nt])

                for m in range(NO):
                    ps_y = psy_pool.tile([P, NT], f32, name=f"psy{t}_{m}", tag="psy")
                    for k in range(K2):
                        nc.tensor.matmul(
                            ps_y[:, :Nt],
                            w2_sb[k][:, ts(m, P)],
                            uT[:, k, :Nt],
                            start=(k == 0),
                            stop=(k == K2 - 1),
                        )
                    y_t = epool.tile([P, NT], f32, name=f"y{t}_{m}", tag="y")
                    nc.scalar.activation(
                        y_t[:, :Nt], ps_y[:, :Nt], AF.Identity,
                        bias=b2_sb[:, m:m + 1],
                    )
                    nc.sync.dma_start(
                        out=out[m * P:(m + 1) * P, off:off + Nt],
                        in_=y_t[:, :Nt],
                    )

    nc.compile()
    return nc


def _route_tokens(xf, Wr, temp):
    """Bit-match the reference's router on CPU jax: logits, top-2, softmax."""
    import jax
    import jax.numpy as jnp

    cpu = jax.devices("cpu")[0]
    with jax.default_device(cpu):
        xj = jnp.asarray(xf)
        logits = (xj @ jnp.asarray(Wr)) / jnp.asarray(temp)
        topw, topi = jax.lax.top_k(logits, TOP_K)
        topw = jax.nn.softmax(topw, axis=-1)
    return np.asarray(topi), np.asarray(topw)


def kernel(**inputs) -> np.ndarray:
    global LAST_RESULTS
    from concourse.bass_utils import run_bass_kernel_spmd

    x = np.asarray(inputs["x"], dtype=np.float32)
    Wr = np.asarray(inputs["Wr"], dtype=np.float32)
    temp = np.asarray(inputs["temp"], dtype=np.float32)
    W1 = np.asarray(inputs["W1"], dtype=np.float32)
    b1 = np.asarray(inputs["b1"], dtype=np.float32)
    W2 = np.asarray(inputs["W2"], dtype=np.float32)
    b2 = np.asarray(inputs["b2"], dtype=np.float32)

    B, S, D = x.shape
    T = B * S
    xf = x.reshape(T, D)

    topi, topw = _route_tokens(xf, Wr, temp)

    # Per-expert token lists and combine weights.
    tok_idx = []
    tok_w = []
    for e in range(NUM_EXPERTS):
        mask = topi == e                       # [T, K]
        sel = mask.any(axis=1)
        idx = np.nonzero(sel)[0]
        w = (topw * mask).sum(axis=1)[idx]
        tok_idx.append(idx)
        tok_w.append(w.astype(np.float32))

    max_count = max(len(i) for i in tok_idx)
    C = max(P, ((max_count + P - 1) // P) * P)

    bf16 = ml_dtypes.bfloat16
    in_maps = []
    for e in range(NUM_EXPERTS):
        idx = tok_idx[e]
        xg = np.zeros((C, D), dtype=np.float32)
        xg[: len(idx)] = xf[idx]
        in_maps.append({
            "xgT": np.ascontiguousarray(xg.T).astype(bf16),
            "w1": W1[e].astype(bf16),
            "w2": W2[e].astype(bf16),
            "b1t": np.ascontiguousarray(b1[e].reshape(2 * D_FF // P, P).T),
            "b2t": np.ascontiguousarray(b2[e].reshape(D_MODEL // P, P).T),
        })

    if C not in _NC_CACHE:
        _NC_CACHE[C] = _build_nc(C)
    nc = _NC_CACHE[C]

    trace = bool(os.environ.get("MOE_KERNEL_TRACE"))
    kwargs = {}
    if trace:
        kwargs = dict(trace=True, trace_cores=list(range(N_CORES)))
    res = run_bass_kernel_spmd(nc, in_maps, core_ids=list(range(N_CORES)), **kwargs)
    LAST_RESULTS = res

    out = np.zeros((T, D), dtype=np.float32)
    for e in range(NUM_EXPERTS):
        idx = tok_idx[e]
        if len(idx) == 0:
            continue
        yT = np.asarray(res.results[e]["out"], dtype=np.float32)  # [D, C]
        y = yT[:, : len(idx)].T
        out[idx] += y * tok_w[e][:, None]

    return out.reshape(B, S, D)


# revision 8
# speedup vs baseline: 1.0804x; 1.0804x over previous
"""Trainium2 Bass kernel for AdaptiveMixtureOfExperts (top-2 SwiGLU MoE).

Strategy (expert-parallel, per the sharding hint):
  - Host computes the tiny router (x @ Wr, top-2, softmax) with jax-on-CPU ops
    that bit-match the reference, then "all-to-all"s tokens by routed expert:
    core e receives exactly the tokens routed to expert e (padded to a common
    capacity C), plus expert e's weights (cast to bf16).
  - Each NeuronCore runs the heavy SwiGLU FFN densely on its gathered tokens:
        hT = W1[e].T @ xgT          (ff on partitions, tokens on free dim)
        uT = (a + b1a) * silu(g + b1g)
        yT = W2[e].T @ uT + b2
    All matmuls keep tokens as the moving/free dimension so no on-chip
    transposes are needed; weights are the stationary operands.
  - Host applies the top-2 combine weights and scatter-adds per-expert outputs
    back into the full [B, S, D] output.

Shapes are hardcoded for the problem instance:
  x:[2,2048,1024] f32, Wr:[1024,8], temp:[1], W1:[8,1024,4096], b1:[8,4096],
  W2:[8,2048,1024], b2:[8,1024].  TOP_K=2, 8 experts on 8 cores.
"""

import os

import numpy as np
import ml_dtypes

D_MODEL = 1024
D_FF = 2048
NUM_EXPERTS = 8
TOP_K = 2
P = 128          # partitions
NT = 512         # token tile (moving free dim per matmul)
N_CORES = 8

_NC_CACHE = {}
LAST_RESULTS = None  # test harness introspection


def _build_nc(C: int, use_silu: bool = True):
    """Build the per-core Bass graph for capacity-C token batches.

    use_silu=False decomposes silu into sigmoid+mul (CoreSim has no Silu).
    """
    import concourse.mybir as mybir
    import concourse.tile as tile
    from concourse import bacc
    from concourse.bass import ts

    f32 = mybir.dt.float32
    bf16 = mybir.dt.bfloat16
    AF = mybir.ActivationFunctionType

    K1 = D_MODEL // P          # 8 k-tiles for matmul1
    K2 = D_FF // P             # 16 k-tiles for matmul2
    NF1 = 2 * D_FF // P        # 32 ff tiles of hT
    NO = D_MODEL // P          # 8 out tiles of yT
    n_tok_tiles = (C + NT - 1) // NT

    nc = bacc.Bacc()
    xgT = nc.declare_dram_parameter("xgT", [D_MODEL, C], bf16, isOutput=False)
    w1 = nc.declare_dram_parameter("w1", [D_MODEL, 2 * D_FF], bf16, isOutput=False)
    w2 = nc.declare_dram_parameter("w2", [D_FF, D_MODEL], bf16, isOutput=False)
    b1t = nc.declare_dram_parameter("b1t", [P, NF1], f32, isOutput=False)
    b2t = nc.declare_dram_parameter("b2t", [P, NO], f32, isOutput=False)
    out = nc.declare_dram_parameter("out", [D_MODEL, C], f32, isOutput=True)

    # DMA chunking: split big loads into per-(k, column-chunk) pieces so the
    # 8 HWDGE queues fill SBUF in the order the PE consumes it.
    WCHUNK = 512

    with tile.TileContext(nc) as tc:
        with (
            tc.tile_pool(name="weights", bufs=1) as wpool,
            tc.tile_pool(name="acts", bufs=2) as upool,
            tc.tile_pool(name="epilogue", bufs=4) as epool,
            tc.tile_pool(name="psA", bufs=2, space="PSUM") as psa_pool,
            tc.tile_pool(name="psG", bufs=2, space="PSUM") as psg_pool,
            tc.tile_pool(name="psY", bufs=2, space="PSUM") as psy_pool,
        ):
            # ---- resident inputs (emitted in PE consumption order) ----
            b1_sb = wpool.tile([P, NF1], f32, name="b1_sb")
            nc.sync.dma_start(out=b1_sb[:], in_=b1t[:])
            b2_sb = wpool.tile([P, NO], f32, name="b2_sb")
            nc.sync.dma_start(out=b2_sb[:], in_=b2t[:])

            xg_sb = [
                wpool.tile([P, C], bf16, name=f"xg_sb{k}", tag=f"xg{k}")
                for k in range(K1)
            ]
            w1_sb = [
                wpool.tile([P, 2 * D_FF], bf16, name=f"w1_sb{k}", tag=f"w1{k}")
                for k in range(K1)
            ]
            w2_sb = [
                wpool.tile([P, D_MODEL], bf16, name=f"w2_sb{k}", tag=f"w2{k}")
                for k in range(K2)
            ]
            # tokens for tile 0 first, then w1 column chunks in consumption
            # order (host pre-interleaves a/g blocks), then w2, then the
            # remaining token tiles.
            for k in range(K1):
                nc.sync.dma_start(
                    out=xg_sb[k][:, :min(NT, C)],
                    in_=xgT[k * P:(k + 1) * P, :min(NT, C)],
                )
            for c0 in range(0, 2 * D_FF, WCHUNK):
                for k in range(K1):
                    nc.sync.dma_start(
                        out=w1_sb[k][:, c0:c0 + WCHUNK],
                        in_=w1[k * P:(k + 1) * P, c0:c0 + WCHUNK],
                    )
            for k in range(K2):
                for c0 in range(0, D_MODEL, WCHUNK):
                    nc.sync.dma_start(
                        out=w2_sb[k][:, c0:c0 + WCHUNK],
                        in_=w2[k * P:(k + 1) * P, c0:c0 + WCHUNK],
                    )
            for t in range(1, n_tok_tiles):
                off = t * NT
                Nt = min(NT, C - off)
                for k in range(K1):
                    nc.sync.dma_start(
                        out=xg_sb[k][:, off:off + Nt],
                        in_=xgT[k * P:(k + 1) * P, off:off + Nt],
                    )

            # ---- main loop over token tiles ----
            # W1 columns are host-permuted to [a_0 | g_0 | a_1 | g_1 | ...]
            # (128-col blocks) so the PE reads w1_sb strictly left-to-right.
            for t in range(n_tok_tiles):
                off = t * NT
                Nt = min(NT, C - off)

                uT = upool.tile([P, K2, NT], bf16, name=f"uT{t}", tag="uT")
                for i in range(K2):
                    ps_a = psa_pool.tile([P, NT], f32, name=f"psa{t}_{i}", tag="psa")
                    for k in range(K1):
                        nc.tensor.matmul(
                            ps_a[:, :Nt],
                            w1_sb[k][:, ts(2 * i, P)],
                            xg_sb[k][:, off:off + Nt],
                            start=(k == 0),
                            stop=(k == K1 - 1),
                        )
                    ps_g = psg_pool.tile([P, NT], f32, name=f"psg{t}_{i}", tag="psg")
                    for k in range(K1):
                        nc.tensor.matmul(
                            ps_g[:, :Nt],
                            w1_sb[k][:, ts(2 * i + 1, P)],
                            xg_sb[k][:, off:off + Nt],
                            start=(k == 0),
                            stop=(k == K1 - 1),
                        )
                    a_t = epool.tile([P, NT], bf16, name=f"a{t}_{i}", tag="a")
                    nc.scalar.activation(
                        a_t[:, :Nt], ps_a[:, :Nt], AF.Identity,
                        bias=b1_sb[:, 2 * i:2 * i + 1],
                    )
                    g_t = epool.tile([P, NT], bf16, name=f"g{t}_{i}", tag="g")
                    if use_silu:
                        nc.scalar.activation(
                            g_t[:, :Nt], ps_g[:, :Nt], AF.Silu,
                            bias=b1_sb[:, 2 * i + 1:2 * i + 2],
                        )
                    else:
                        s_t = epool.tile([P, NT], bf16, name=f"s{t}_{i}", tag="s")
                        nc.scalar.activation(
                            s_t[:, :Nt], ps_g[:, :Nt], AF.Sigmoid,
                            bias=b1_sb[:, 2 * i + 1:2 * i + 2],
                        )
                        gb_t = epool.tile([P, NT], bf16, name=f"gb{t}_{i}", tag="gb")
                        nc.scalar.activation(
                            gb_t[:, :Nt], ps_g[:, :Nt], AF.Identity,
                            bias=b1_sb[:, 2 * i + 1:2 * i + 2],
                        )
                        nc.vector.tensor_mul(g_t[:, :Nt], gb_t[:, :Nt], s_t[:, :Nt])
                    nc.vector.tensor_mul(uT[:, i, :Nt], a_t[:, :Nt], g_t[:, :Nt])

                for m in range(NO):
                    ps_y = psy_pool.tile([P, NT], f32, name=f"psy{t}_{m}", tag="psy")
                    for k in range(K2):
                        nc.tensor.matmul(
                            ps_y[:, :Nt],
                            w2_sb[k][:, ts(m, P)],
                            uT[:, k, :Nt],
                            start=(k == 0),
                            stop=(k == K2 - 1),
                        )
                    y_t = epool.tile([P, NT], f32, name=f"y{t}_{m}", tag="y")
                    nc.scalar.activation(
                        y_t[:, :Nt], ps_y[:, :Nt], AF.Identity,
                        bias=b2_sb[:, m:m + 1],
                    )
                    nc.sync.dma_start(
                        out=out[m * P:(m + 1) * P, off:off + Nt],
                        in_=y_t[:, :Nt],
                    )

    nc.compile()
    return nc


def _route_tokens(xf, Wr, temp):
    """Bit-match the reference's router on CPU jax: logits, top-2, softmax."""
    import jax
    import jax.numpy as jnp

    cpu = jax.devices("cpu")[0]
    with jax.default_device(cpu):
        xj = jnp.asarray(xf)
        logits = (xj @ jnp.asarray(Wr)) / jnp.asarray(temp)
        topw, topi = jax.lax.top_k(logits, TOP_K)
        topw = jax.nn.softmax(topw, axis=-1)
    return np.asarray(topi), np.asarray(topw)


def kernel(**inputs) -> np.ndarray:
    global LAST_RESULTS
    from concourse.bass_utils import run_bass_kernel_spmd

    x = np.asarray(inputs["x"], dtype=np.float32)
    Wr = np.asarray(inputs["Wr"], dtype=np.float32)
    temp = np.asarray(inputs["temp"], dtype=np.float32)
    W1 = np.asarray(inputs["W1"], dtype=np.float32)
    b1 = np.asarray(inputs["b1"], dtype=np.float32)
    W2 = np.asarray(inputs["W2"], dtype=np.float32)
    b2 = np.asarray(inputs["b2"], dtype=np.float32)

    B, S, D = x.shape
    T = B * S
    xf = x.reshape(T, D)

    topi, topw = _route_tokens(xf, Wr, temp)

    # Per-expert token lists and combine weights.
    tok_idx = []
    tok_w = []
    for e in range(NUM_EXPERTS):
        mask = topi == e                       # [T, K]
        sel = mask.any(axis=1)
        idx = np.nonzero(sel)[0]
        w = (topw * mask).sum(axis=1)[idx]
        tok_idx.append(idx)
        tok_w.append(w.astype(np.float32))

    max_count = max(len(i) for i in tok_idx)
    C = max(P, ((max_count + 31) // 32) * 32)

    # Interleave W1's a/g column blocks ([a_0|g_0|a_1|g_1|...], 128-col
    # blocks) so the device consumes w1 columns strictly left-to-right.
    blk = np.arange(2 * D_FF).reshape(2, D_FF // P, P)   # [a/g, block, col]
    perm = blk.transpose(1, 0, 2).reshape(-1)

    bf16 = ml_dtypes.bfloat16
    in_maps = []
    for e in range(NUM_EXPERTS):
        idx = tok_idx[e]
        xg = np.zeros((C, D), dtype=np.float32)
        xg[: len(idx)] = xf[idx]
        b1p = b1[e][perm]
        in_maps.append({
            "xgT": np.ascontiguousarray(xg.T).astype(bf16),
            "w1": np.ascontiguousarray(W1[e][:, perm]).astype(bf16),
            "w2": W2[e].astype(bf16),
            "b1t": np.ascontiguousarray(b1p.reshape(2 * D_FF // P, P).T),
            "b2t": np.ascontiguousarray(b2[e].reshape(D_MODEL // P, P).T),
        })

    if C not in _NC_CACHE:
        _NC_CACHE[C] = _build_nc(C)
    nc = _NC_CACHE[C]

    trace = bool(os.environ.get("MOE_KERNEL_TRACE"))
    kwargs = {}
    if trace:
        kwargs = dict(trace=True, trace_cores=list(range(N_CORES)))
    res = run_bass_kernel_spmd(nc, in_maps, core_ids=list(range(N_CORES)), **kwargs)
    LAST_RESULTS = res

    out = np.zeros((T, D), dtype=np.float32)
    for e in range(NUM_EXPERTS):
        idx = tok_idx[e]
        if len(idx) == 0:
            continue
        yT = np.asarray(res.results[e]["out"], dtype=np.float32)  # [D, C]
        y = yT[:, : len(idx)].T
        out[idx] += y * tok_w[e][:, None]

    return out.reshape(B, S, D)
